# revision 1
# baseline (speedup 1.0000x reference)
"""BitNetAttention Trainium2 kernel (nn_BitNetAttention, B=2 S=2048 HID=2560).

Reference: q/k/v projections (x @ W^T), RoPE (rotate-half, theta=5e5), causal
GQA attention (20 q heads, 5 kv heads, head_dim 128), BitNetSubNorm per-channel
gain, o_proj.

Sharding across 8 NeuronCores: core c handles batch c//4 and 5 query heads:
with g = c%4, q heads [4g..4g+3, 16+g], kv heads [g, 4]. This grouping makes
the local head->kv map the constant [0,0,0,0,1] so one SPMD program serves all
cores. Each core computes its batch's partial o_proj output (sum over its 5
heads); the host sums 4 partials per batch. sub_w is folded into wo on host.

Precision: everything bf16 with fp32 PSUM accumulation; RoPE in fp32 from
bf16 cos/sin tables; softmax normalizer 1/R computed as exp(-ln R) on the
ACT engine (keeps the slow DVE reciprocal off the per-head critical path;
bass blocks the ACT Reciprocal function and custom-DVE approx ops fail
walrus codegen here).

Structure: one pass over 512-wide x blocks; for each block compute K strips
(RoPE'd, [d,t] layout), V tiles, Q strips (V PSUM groups interleaved between
K/Q strips so their single-buffer copy hides under strip matmuls), then
immediately the attention chunk for the same 512 queries (QK^T in [k,q]
layout -> exp on ACT -> causal mask mul on the diagonal tile -> PV and
ones-matmul row sums accumulating in PSUM) followed by that chunk's o_proj
(PSUM shared with QK scores via the 4-deep work pool), staged to bf16 via
DVE and DMA'd out 1024 wide. Interleaving projections with attention keeps
the PE fed across phase boundaries (PE ~90% busy). Walrus's 1-sync-wait ISA
limit is handled by a post-pass moving surplus waits onto EventSemaphore
instructions.
"""

import numpy as np
import ml_dtypes
from contextlib import ExitStack

import concourse.bass as bass
import concourse.mybir as mybir
import concourse.tile as tile
from concourse.bass_utils import run_bass_kernel_spmd

F32 = mybir.dt.float32
BF16 = mybir.dt.bfloat16

B, S, HID = 2, 2048, 2560
NH, NKV, HD = 20, 5, 128
G = NH // NKV
THETA = 500000.0
NCORES = 8
HEADS = 5          # query heads per core
KV = 2             # kv heads per core
KVIDX = [0, 0, 0, 0, 1]   # local head -> local kv head
HT = HID // 128    # 20 hidden k-tiles
BLK = 512          # x block width (t) = attention q-chunk width
NBLK = S // BLK    # 4
CH = 512
KT = S // 128      # 16 k-tiles
NO = HID // CH     # 5 o_proj hid chunks
SCALE = HD ** -0.5

_CACHE = {}


def _split_waits(nc):
    """Walrus ISA structs carry a single sync-wait slot. Move surplus waits
    onto EventSemaphore sequencer instructions inserted just before (same
    engine; engines are in-order so hoisting waits earlier is safe)."""
    import concourse.mybir as mb
    n_ev = 0
    for f in nc.m.functions:
        for bb in f.blocks:
            out = []
            changed = False
            for inst in bb.instructions:
                si = getattr(inst, "sync_info", None)
                if (type(inst).__name__ != "InstEventSemaphore" and si is not None
                        and len(si.on_wait) > 1):
                    waits = list(si.on_wait)
                    for w in waits[:-1]:
                        ev = mb.InstEventSemaphore(name=f"I-evw-{n_ev}", ins=[], outs=[])
                        n_ev += 1
                        ev.engine = inst.engine
                        ev.sync_info = mb.SyncInfo(on_wait=[w], on_update=[])
                        nc.register_instruction(ev)
                        out.append(ev)
                    inst.sync_info = mb.SyncInfo(on_wait=waits[-1:],
                                                 on_update=list(si.on_update))
                    changed = True
                out.append(inst)
            if changed:
                bb.instructions = out
    return n_ev


def build_nc(reps=1):
    nc = bass.Bass()
    xT = nc.declare_dram_parameter("xT", [HID, S], BF16, isOutput=False)
    wqT = nc.declare_dram_parameter("wqT", [HID, HEADS * HD], BF16, isOutput=False)
    wkT = nc.declare_dram_parameter("wkT", [HID, KV * HD], BF16, isOutput=False)
    wvT = nc.declare_dram_parameter("wvT", [HID, KV * HD], BF16, isOutput=False)
    woT = nc.declare_dram_parameter("woT", [HEADS * HD, HID], BF16, isOutput=False)
    cosT = nc.declare_dram_parameter("cosT", [HD, S], BF16, isOutput=False)
    sinT = nc.declare_dram_parameter("sinT", [HD, S], BF16, isOutput=False)  # sign-folded
    mask = nc.declare_dram_parameter("mask", [128, 128], BF16, isOutput=False)  # triu 0/1
    ones = nc.declare_dram_parameter("ones", [128, 128], BF16, isOutput=False)
    y = nc.declare_dram_parameter("y", [S, HID], BF16, isOutput=True)

    xT_t = xT.rearrange("(a p) t -> p a t", p=128)      # [128, 20, 2048]
    wqT_t = wqT.rearrange("(a p) d -> p a d", p=128)    # [128, 20, 640]
    wkT_t = wkT.rearrange("(a p) d -> p a d", p=128)
    wvT_t = wvT.rearrange("(a p) d -> p a d", p=128)
    woT_t = woT.rearrange("(h p) n -> p h n", p=128)    # [128, 5, 2560]

    with tile.TileContext(nc) as tc:
      for rep in range(reps):
        with ExitStack() as ctx:
            # ---------- persistent tiles ----------
            per = ctx.enter_context(tc.tile_pool(name=f"persist{rep}", bufs=1))
            kT_sb = per.tile([128, KV, S], BF16)        # 8KB/part
            v_sb = per.tile([128, KT, KV * HD], BF16)   # 8KB/part [t%128, ktile, kv*128+d]
            cos_sb = per.tile([HD, S], BF16)            # 4KB
            sin_sb = per.tile([HD, S], BF16)            # 4KB
            mask_sb = per.tile([128, 128], BF16)
            ones_sb = per.tile([128, 128], BF16)
            qT_sb = per.tile([128, HEADS, S], BF16)     # 20KB/part

            wp = ctx.enter_context(tc.tile_pool(name=f"w{rep}", bufs=1))
            wk_sb = wp.tile([128, HT, KV * HD], BF16)   # 10KB/part
            wv_sb = wp.tile([128, HT, KV * HD], BF16)   # 10KB/part
            wq_sb = wp.tile([128, HT, HEADS * HD], BF16)  # 25.6KB/part
            wo_sb = wp.tile([128, HEADS, HID], BF16)    # 25.6KB/part

            xp = ctx.enter_context(tc.tile_pool(name=f"x{rep}", bufs=2))
            ropep = ctx.enter_context(tc.tile_pool(name=f"rope{rep}", bufs=1))
            # PSUM pools (8 banks total): work pool (proj strips + QK
            # scores + o_proj py) 4 banks, pv 1, pat 2, pR 1
            wkpp = ctx.enter_context(tc.tile_pool(name=f"pw{rep}", bufs=4, space="PSUM"))
            pvp = ctx.enter_context(tc.tile_pool(name=f"pv{rep}", bufs=1, space="PSUM"))
            patp = ctx.enter_context(tc.tile_pool(name=f"pat{rep}", bufs=2, space="PSUM"))
            prp = ctx.enter_context(tc.tile_pool(name=f"pr{rep}", bufs=1, space="PSUM"))
            etp = ctx.enter_context(tc.tile_pool(name=f"et{rep}", bufs=4))
            recp = ctx.enter_context(tc.tile_pool(name=f"rec{rep}", bufs=2))
            atp = ctx.enter_context(tc.tile_pool(name=f"at{rep}", bufs=2))
            cst = ctx.enter_context(tc.tile_pool(name=f"yst{rep}", bufs=3))

            def rope(dst, psrc, t0, w):
                t1 = ropep.tile([128, BLK], F32, tag="t1")
                t2 = ropep.tile([128, BLK], F32, tag="t2")
                nc.vector.tensor_mul(t1[:, 0:w], psrc, cos_sb[:, t0:t0 + w])
                nc.vector.tensor_mul(t2[0:64, 0:w], psrc[64:128, :], sin_sb[0:64, t0:t0 + w])
                nc.vector.tensor_mul(t2[64:128, 0:w], psrc[0:64, :], sin_sb[64:128, t0:t0 + w])
                nc.vector.tensor_add(dst, t1[:, 0:w], t2[:, 0:w])

            # xblk as 4 independent tiles so the first K matmuls only wait
            # on the first 5 a-tiles, not the whole block. blk0's tiles load
            # inline (interleaved with wk); later blocks are prefetched at
            # the end of the PREVIOUS block's projections, ahead of that
            # block's y-output DMAs on the Sync queue.
            def alloc_xq():
                return [xp.tile([128, 5, BLK], BF16, tag=f"xb{q}", name=f"xb{q}")
                        for q in range(4)]

            def load_xq(tiles, blk_):
                tb = blk_ * BLK
                for q in range(4):
                    nc.sync.dma_start(out=tiles[q],
                                      in_=xT_t[:, 5 * q:5 * (q + 1), tb:tb + BLK])

            xq = None
            for blk in range(NBLK):
                t0 = blk * BLK
                if blk == 0:
                    xq = alloc_xq()
                    for q in range(4):
                        if q == 0:
                            # finest-grain first tiles so the first K matmul
                            # can start as early as possible
                            nc.sync.dma_start(out=wk_sb[:, 0:1, :], in_=wkT_t[:, 0:1, :])
                            nc.sync.dma_start(out=xq[0][:, 0:1, :],
                                              in_=xT_t[:, 0:1, t0:t0 + BLK])
                            nc.sync.dma_start(out=wk_sb[:, 1:5, :], in_=wkT_t[:, 1:5, :])
                            nc.sync.dma_start(out=xq[0][:, 1:5, :],
                                              in_=xT_t[:, 1:5, t0:t0 + BLK])
                            continue
                        nc.sync.dma_start(out=wk_sb[:, 5 * q:5 * (q + 1), :],
                                          in_=wkT_t[:, 5 * q:5 * (q + 1), :])
                        nc.sync.dma_start(out=xq[q],
                                          in_=xT_t[:, 5 * q:5 * (q + 1), t0:t0 + BLK])
                        if q == 1:
                            # in time for the first K strip's RoPE, but out of
                            # the way of the a<=9 portion of the chain
                            nc.sync.dma_start(out=cos_sb[:, 0:BLK], in_=cosT[:, 0:BLK])
                            nc.sync.dma_start(out=sin_sb[:, 0:BLK], in_=sinT[:, 0:BLK])

                def xblk(a, sl=slice(None), xq_=xq):
                    return xq_[a // 5][:, a % 5, sl]

                if blk == 0:
                    for q in range(4):
                        nc.sync.dma_start(out=wv_sb[:, 5 * q:5 * (q + 1), :],
                                          in_=wvT_t[:, 5 * q:5 * (q + 1), :])
                    nc.sync.dma_start(out=wq_sb, in_=wqT_t)
                    nc.sync.dma_start(out=mask_sb, in_=mask[:])
                    nc.sync.dma_start(out=ones_sb, in_=ones[:])
                    nc.sync.dma_start(out=cos_sb[:, BLK:S], in_=cosT[:, BLK:S])
                    nc.sync.dma_start(out=sin_sb[:, BLK:S], in_=sinT[:, BLK:S])
                    nc.sync.dma_start(out=wo_sb, in_=woT_t)

                # ---- K/V/Q projections, V groups interleaved between K/Q
                # strips so each pv PSUM copy (pvp bufs=1) hides under the
                # following strip's matmuls ----
                def kstrip(kvh):
                    pk = wkpp.tile([128, BLK], F32, tag="ps", name="pk")
                    for a in range(HT):
                        nc.tensor.matmul(pk[:], wk_sb[:, a, kvh * HD:(kvh + 1) * HD],
                                         xblk(a), start=(a == 0), stop=(a == HT - 1))
                    rope(kT_sb[:, kvh, t0:t0 + BLK], pk[:], t0, BLK)

                def vgroup(tt):
                    pv = pvp.tile([128, KV * HD], F32, tag="pv", name="pv")
                    for a in range(HT):
                        nc.tensor.matmul(pv[:], xblk(a, slice(tt * 128, (tt + 1) * 128)),
                                         wv_sb[:, a, :], start=(a == 0), stop=(a == HT - 1))
                    nc.scalar.copy(v_sb[:, blk * (BLK // 128) + tt, :], pv[:])

                def qstrip(h):
                    pq = wkpp.tile([128, BLK], F32, tag="ps", name="pq")
                    for a in range(HT):
                        nc.tensor.matmul(pq[:], wq_sb[:, a, h * HD:(h + 1) * HD],
                                         xblk(a), start=(a == 0), stop=(a == HT - 1))
                    rope(qT_sb[:, h, t0:t0 + BLK], pq[:], t0, BLK)

                if blk == 0:
                    # wv/wq are still in flight behind the wk/xb stream:
                    # front-load both K strips (wk+xb only) to keep the PE
                    # busy until wv lands, then V groups until wq lands
                    kstrip(0); kstrip(1); vgroup(0); qstrip(0); vgroup(1)
                    qstrip(1); vgroup(2); qstrip(2); vgroup(3); qstrip(3); qstrip(4)
                else:
                    kstrip(0); vgroup(0); kstrip(1); vgroup(1); qstrip(0)
                    vgroup(2); qstrip(1); vgroup(3); qstrip(2); qstrip(3); qstrip(4)

                # prefetch the next block's x ahead of this chunk's y DMAs
                if blk + 1 < NBLK:
                    xq_next = alloc_xq()
                    load_xq(xq_next, blk + 1)

                # ---- attention chunk for these 512 queries ----
                c = blk
                q0 = c * CH
                ki_max = 4 * c + 3
                at_ch = atp.tile([128, HEADS, CH], BF16, tag="atc")  # 5KB/part
                for h in range(HEADS):
                    kvh = KVIDX[h]
                    pR = prp.tile([128, CH], F32, tag="pR")
                    pat = patp.tile([128, CH], F32, tag="pat")
                    for ki in range(ki_max + 1):
                        off = max(0, ki * 128 - q0)
                        ps = wkpp.tile([128, CH], F32, tag="ps")
                        nc.tensor.matmul(ps[:, off:CH],
                                         kT_sb[:, kvh, ki * 128:(ki + 1) * 128],
                                         qT_sb[:, h, q0 + off:q0 + CH],
                                         start=True, stop=True)
                        et = etp.tile([128, CH], BF16, tag="et")
                        nc.scalar.activation(out=et[:, off:CH], in_=ps[:, off:CH],
                                             func=mybir.ActivationFunctionType.Exp,
                                             scale=SCALE)
                        if ki >= 4 * c:  # diagonal tile: causal mask
                            nc.vector.tensor_mul(et[:, off:off + 128],
                                                 et[:, off:off + 128], mask_sb[:])
                        # pR before PV: at the head tail, ln(pR) is the
                        # critical path to recycling pat/pR, so start it one
                        # matmul earlier
                        nc.tensor.matmul(pR[:, off:CH], ones_sb[:], et[:, off:CH],
                                         start=(ki == 0), stop=(ki == ki_max))
                        nc.tensor.matmul(pat[:, off:CH],
                                         v_sb[:, ki, kvh * HD:(kvh + 1) * HD],
                                         et[:, off:CH], start=(ki == 0),
                                         stop=(ki == ki_max))
                    # 1/R = exp(-ln R) on the ACT engine: keeps the slow DVE
                    # reciprocal off the per-head critical path.
                    lr = recp.tile([128, CH], F32, tag="lr")
                    nc.scalar.activation(out=lr[:], in_=pR[:],
                                         func=mybir.ActivationFunctionType.Ln)
                    rec = recp.tile([128, CH], F32, tag="rec")
                    nc.scalar.activation(out=rec[:], in_=lr[:],
                                         func=mybir.ActivationFunctionType.Exp,
                                         scale=-1.0)
                    nc.vector.tensor_mul(at_ch[:, h, :], pat[:], rec[:])
                # ---- o_proj for this chunk (bf16 out, ACT copies) ----
                for tt in range(CH // 128):
                    for n0 in range(0, NO, 2):
                        nn_ = min(2, NO - n0)
                        yst = cst.tile([128, 2 * CH], BF16, tag="yst")
                        for j in range(nn_):
                            n = n0 + j
                            py = wkpp.tile([128, CH], F32, tag="ps")
                            for h in range(HEADS):
                                nc.tensor.matmul(py[:], at_ch[:, h, tt * 128:(tt + 1) * 128],
                                                 wo_sb[:, h, n * CH:(n + 1) * CH],
                                                 start=(h == 0), stop=(h == HEADS - 1))
                            nc.vector.tensor_copy(yst[:, j * CH:(j + 1) * CH], py[:])
                        nc.sync.dma_start(
                            out=y[q0 + tt * 128:q0 + (tt + 1) * 128,
                                  n0 * CH:(n0 + nn_) * CH],
                            in_=yst[:, 0:nn_ * CH])
                if blk + 1 < NBLK:
                    xq = xq_next

    _split_waits(nc)
    nc.finalize()
    return nc


def core_heads(g):
    """Query-head and kv-head global indices for core group g (= core % 4)."""
    qh = [4 * g, 4 * g + 1, 4 * g + 2, 4 * g + 3, 16 + g]
    kvh = [g, 4]
    return qh, kvh


def make_in_maps(hidden_states, position_ids, wq, wk, wv, wo, sub_w):
    hidden_states = np.asarray(hidden_states, dtype=np.float32)
    position_ids = np.asarray(position_ids)
    wq = np.asarray(wq, dtype=np.float32)
    wk = np.asarray(wk, dtype=np.float32)
    wv = np.asarray(wv, dtype=np.float32)
    wo = np.asarray(wo, dtype=np.float32)
    sub_w = np.asarray(sub_w, dtype=np.float32)

    wo_s = wo * sub_w[None, :]          # fold BitNetSubNorm gain into o_proj
    inv_freq = (1.0 / (THETA ** (np.arange(0, HD, 2, dtype=np.float32) / HD)))  # [64]
    mask01 = np.triu(np.ones((128, 128))).astype(ml_dtypes.bfloat16)
    bf = ml_dtypes.bfloat16
    ones128 = np.ones((128, 128), dtype=bf)

    # per-batch tensors (shared by the 4 cores of a batch): compute once
    xT_b, cosT_b, sinT_b = [], [], []
    for b in range(B):
        xT_b.append(np.ascontiguousarray(hidden_states[b].T).astype(bf))  # [HID, S]
        pos = position_ids[b].astype(np.float32)                      # [S]
        ang = inv_freq[:, None] * pos[None, :]                        # [64, S]
        cosT = np.concatenate([np.cos(ang), np.cos(ang)], axis=0)     # [128, S]
        sinT = np.concatenate([-np.sin(ang), np.sin(ang)], axis=0)    # sign-folded
        cosT_b.append(np.ascontiguousarray(cosT).astype(bf))
        sinT_b.append(np.ascontiguousarray(sinT).astype(bf))
    # per-group weight shards (shared by the 2 cores of a group): once
    wshard = []
    for g in range(4):
        qh, kvh = core_heads(g)
        qrows = np.concatenate([np.arange(h * HD, (h + 1) * HD) for h in qh])
        krows = np.concatenate([np.arange(k * HD, (k + 1) * HD) for k in kvh])
        wshard.append({
            "wqT": np.ascontiguousarray(wq[qrows].T).astype(bf),      # [HID, 640]
            "wkT": np.ascontiguousarray(wk[krows].T).astype(bf),      # [HID, 256]
            "wvT": np.ascontiguousarray(wv[krows].T).astype(bf),      # [HID, 256]
            "woT": np.ascontiguousarray(wo_s[:, qrows].T).astype(bf),
        })

    in_maps = []
    for c in range(NCORES):
        b, g = c // 4, c % 4
        in_maps.append({
            "xT": xT_b[b],
            "cosT": cosT_b[b],
            "sinT": sinT_b[b],
            "mask": mask01,
            "ones": ones128,
            **wshard[g],
        })
    return in_maps


def kernel(hidden_states, position_ids, wq, wk, wv, wo, sub_w, _trace=False):
    if "nc" not in _CACHE:
        _CACHE["nc"] = build_nc()
    nc = _CACHE["nc"]
    in_maps = make_in_maps(hidden_states, position_ids, wq, wk, wv, wo, sub_w)
    res = run_bass_kernel_spmd(nc, in_maps, core_ids=list(range(NCORES)), trace=_trace)
    _CACHE["last_results"] = res
    out = np.zeros((B, S, HID), dtype=np.float32)
    for c in range(NCORES):
        out[c // 4] += res.results[c]["y"].astype(np.float32)
    return out



# revision 6
# speedup vs baseline: 1.3349x; 1.3349x over previous
"""BitNetAttention Trainium2 kernel (nn_BitNetAttention, B=2 S=2048 HID=2560).

Reference: q/k/v projections (x @ W^T), RoPE (rotate-half, theta=5e5), causal
GQA attention (20 q heads, 5 kv heads, head_dim 128), BitNetSubNorm per-channel
gain, o_proj.

Sharding across 8 NeuronCores: core c handles batch c//4 and 5 query heads:
with g = c%4, q heads [4g..4g+3, 16+g], kv heads [g, 4]. This grouping makes
the local head->kv map the constant [0,0,0,0,1] so one SPMD program serves all
cores. Each core computes its batch's partial o_proj output (sum over its 5
heads); the host sums 4 partials per batch. sub_w is folded into wo on host.

Precision: everything bf16 with fp32 PSUM accumulation; RoPE in fp32 from
bf16 cos/sin tables; softmax normalizer 1/R computed as exp(-ln R) on the
ACT engine (keeps the slow DVE reciprocal off the per-head critical path;
bass blocks the ACT Reciprocal function and custom-DVE approx ops fail
walrus codegen here).

Structure: one pass over 512-wide x blocks; for each block compute K strips
(RoPE'd, [d,t] layout), V tiles, Q strips (V PSUM groups interleaved between
K/Q strips so their single-buffer copy hides under strip matmuls), then
immediately the attention chunk for the same 512 queries (QK^T in [k,q]
layout -> exp on ACT -> causal mask mul on the diagonal tile -> PV
accumulating in PSUM). Softmax row sums accumulate on the DVE (fp32 SBUF
tile += exp tile per ki, final partial written bf16) with a single
ones-matmul per (head, chunk) reducing over partitions -- this keeps the
per-ki row-sum matmuls off the PE (~36us/core saved vs accumulating R in
PSUM). The NEXT block's projections are issued before this chunk's o_proj
so the PE has proj work while the ACT drains the chunk's exp tail; o_proj
(PSUM shared with QK scores via the 4-deep work pool) stages to bf16 via
DVE and DMAs out 1024 wide. PE ~90% busy. Walrus's 1-sync-wait ISA limit is
handled by a post-pass moving surplus waits onto EventSemaphore
instructions.
"""

import numpy as np
import ml_dtypes
from contextlib import ExitStack

import concourse.bass as bass
import concourse.mybir as mybir
import concourse.tile as tile
from concourse.bass_utils import run_bass_kernel_spmd

F32 = mybir.dt.float32
BF16 = mybir.dt.bfloat16

B, S, HID = 2, 2048, 2560
NH, NKV, HD = 20, 5, 128
G = NH // NKV
THETA = 500000.0
NCORES = 8
HEADS = 5          # query heads per core
KV = 2             # kv heads per core
KVIDX = [0, 0, 0, 0, 1]   # local head -> local kv head
HT = HID // 128    # 20 hidden k-tiles
BLK = 512          # x block width (t) = attention q-chunk width
NBLK = S // BLK    # 4
CH = 512
KT = S // 128      # 16 k-tiles
NO = HID // CH     # 5 o_proj hid chunks
SCALE = HD ** -0.5

_CACHE = {}


def _split_waits(nc):
    """Walrus ISA structs carry a single sync-wait slot. Move surplus waits
    onto EventSemaphore sequencer instructions inserted just before (same
    engine; engines are in-order so hoisting waits earlier is safe)."""
    import concourse.mybir as mb
    n_ev = 0
    for f in nc.m.functions:
        for bb in f.blocks:
            out = []
            changed = False
            for inst in bb.instructions:
                si = getattr(inst, "sync_info", None)
                if (type(inst).__name__ != "InstEventSemaphore" and si is not None
                        and len(si.on_wait) > 1):
                    waits = list(si.on_wait)
                    for w in waits[:-1]:
                        ev = mb.InstEventSemaphore(name=f"I-evw-{n_ev}", ins=[], outs=[])
                        n_ev += 1
                        ev.engine = inst.engine
                        ev.sync_info = mb.SyncInfo(on_wait=[w], on_update=[])
                        nc.register_instruction(ev)
                        out.append(ev)
                    inst.sync_info = mb.SyncInfo(on_wait=waits[-1:],
                                                 on_update=list(si.on_update))
                    changed = True
                out.append(inst)
            if changed:
                bb.instructions = out
    return n_ev


def build_nc(reps=1):
    nc = bass.Bass()
    xT = nc.declare_dram_parameter("xT", [HID, S], BF16, isOutput=False)
    wqT = nc.declare_dram_parameter("wqT", [HID, HEADS * HD], BF16, isOutput=False)
    wkT = nc.declare_dram_parameter("wkT", [HID, KV * HD], BF16, isOutput=False)
    wvT = nc.declare_dram_parameter("wvT", [HID, KV * HD], BF16, isOutput=False)
    woT = nc.declare_dram_parameter("woT", [HEADS * HD, HID], BF16, isOutput=False)
    cosT = nc.declare_dram_parameter("cosT", [HD, S], BF16, isOutput=False)
    sinT = nc.declare_dram_parameter("sinT", [HD, S], BF16, isOutput=False)  # sign-folded
    mask = nc.declare_dram_parameter("mask", [128, 128], BF16, isOutput=False)  # triu 0/1
    ones = nc.declare_dram_parameter("ones", [128, 128], BF16, isOutput=False)
    y = nc.declare_dram_parameter("y", [S, HID], BF16, isOutput=True)

    xT_t = xT.rearrange("(a p) t -> p a t", p=128)      # [128, 20, 2048]
    wqT_t = wqT.rearrange("(a p) d -> p a d", p=128)    # [128, 20, 640]
    wkT_t = wkT.rearrange("(a p) d -> p a d", p=128)
    wvT_t = wvT.rearrange("(a p) d -> p a d", p=128)
    woT_t = woT.rearrange("(h p) n -> p h n", p=128)    # [128, 5, 2560]

    with tile.TileContext(nc) as tc:
      for rep in range(reps):
        with ExitStack() as ctx:
            # ---------- persistent tiles ----------
            per = ctx.enter_context(tc.tile_pool(name=f"persist{rep}", bufs=1))
            kT_sb = per.tile([128, KV, S], BF16)        # 8KB/part
            v_sb = per.tile([128, KT, KV * HD], BF16)   # 8KB/part [t%128, ktile, kv*128+d]
            cos_sb = per.tile([HD, S], BF16)            # 4KB
            sin_sb = per.tile([HD, S], BF16)            # 4KB
            mask_sb = per.tile([128, 128], BF16)
            ones_sb = per.tile([128, 128], BF16)
            qT_sb = per.tile([128, HEADS, S], BF16)     # 20KB/part

            wp = ctx.enter_context(tc.tile_pool(name=f"w{rep}", bufs=1))
            wk_sb = wp.tile([128, HT, KV * HD], BF16)   # 10KB/part
            wv_sb = wp.tile([128, HT, KV * HD], BF16)   # 10KB/part
            wq_sb = wp.tile([128, HT, HEADS * HD], BF16)  # 25.6KB/part
            wo_sb = wp.tile([128, HEADS, HID], BF16)    # 25.6KB/part

            xp = ctx.enter_context(tc.tile_pool(name=f"x{rep}", bufs=2))
            ropep = ctx.enter_context(tc.tile_pool(name=f"rope{rep}", bufs=1))
            # PSUM pools (8 banks total): work pool (proj strips + QK
            # scores + o_proj py) 4 banks, pv 1, pat 2, pR 1
            wkpp = ctx.enter_context(tc.tile_pool(name=f"pw{rep}", bufs=4, space="PSUM"))
            pvp = ctx.enter_context(tc.tile_pool(name=f"pv{rep}", bufs=1, space="PSUM"))
            patp = ctx.enter_context(tc.tile_pool(name=f"pat{rep}", bufs=2, space="PSUM"))
            prp = ctx.enter_context(tc.tile_pool(name=f"pr{rep}", bufs=1, space="PSUM"))
            etp = ctx.enter_context(tc.tile_pool(name=f"et{rep}", bufs=4))
            recp = ctx.enter_context(tc.tile_pool(name=f"rec{rep}", bufs=2))
            raccp = ctx.enter_context(tc.tile_pool(name=f"racc{rep}", bufs=2))
            atp = ctx.enter_context(tc.tile_pool(name=f"at{rep}", bufs=2))
            cst = ctx.enter_context(tc.tile_pool(name=f"yst{rep}", bufs=3))

            def rope(dst, psrc, t0, w):
                t1 = ropep.tile([128, BLK], F32, tag="t1")
                t2 = ropep.tile([128, BLK], F32, tag="t2")
                nc.vector.tensor_mul(t1[:, 0:w], psrc, cos_sb[:, t0:t0 + w])
                nc.vector.tensor_mul(t2[0:64, 0:w], psrc[64:128, :], sin_sb[0:64, t0:t0 + w])
                nc.vector.tensor_mul(t2[64:128, 0:w], psrc[0:64, :], sin_sb[64:128, t0:t0 + w])
                nc.vector.tensor_add(dst, t1[:, 0:w], t2[:, 0:w])

            # xblk as 4 independent tiles so the first K matmuls only wait
            # on the first 5 a-tiles, not the whole block. blk0's tiles load
            # inline (interleaved with wk); later blocks are prefetched at
            # the end of the PREVIOUS block's projections, ahead of that
            # block's y-output DMAs on the Sync queue.
            def alloc_xq():
                return [xp.tile([128, 5, BLK], BF16, tag=f"xb{q}", name=f"xb{q}")
                        for q in range(4)]

            def load_xq(tiles, blk_):
                tb = blk_ * BLK
                for q in range(4):
                    nc.sync.dma_start(out=tiles[q],
                                      in_=xT_t[:, 5 * q:5 * (q + 1), tb:tb + BLK])

            xq = None
            for blk in range(NBLK):
                t0 = blk * BLK
                if blk == 0:
                    xq = alloc_xq()
                    for q in range(4):
                        if q == 0:
                            # finest-grain first tiles so the first K matmul
                            # can start as early as possible
                            nc.sync.dma_start(out=wk_sb[:, 0:1, :], in_=wkT_t[:, 0:1, :])
                            nc.sync.dma_start(out=xq[0][:, 0:1, :],
                                              in_=xT_t[:, 0:1, t0:t0 + BLK])
                            nc.sync.dma_start(out=wk_sb[:, 1:5, :], in_=wkT_t[:, 1:5, :])
                            nc.sync.dma_start(out=xq[0][:, 1:5, :],
                                              in_=xT_t[:, 1:5, t0:t0 + BLK])
                            continue
                        nc.sync.dma_start(out=wk_sb[:, 5 * q:5 * (q + 1), :],
                                          in_=wkT_t[:, 5 * q:5 * (q + 1), :])
                        nc.sync.dma_start(out=xq[q],
                                          in_=xT_t[:, 5 * q:5 * (q + 1), t0:t0 + BLK])
                        if q == 1:
                            # in time for the first K strip's RoPE, but out of
                            # the way of the a<=9 portion of the chain
                            nc.sync.dma_start(out=cos_sb[:, 0:BLK], in_=cosT[:, 0:BLK])
                            nc.sync.dma_start(out=sin_sb[:, 0:BLK], in_=sinT[:, 0:BLK])

                def xblk(a, sl=slice(None), xq_=xq):
                    return xq_[a // 5][:, a % 5, sl]

                if blk == 0:
                    for q in range(4):
                        nc.sync.dma_start(out=wv_sb[:, 5 * q:5 * (q + 1), :],
                                          in_=wvT_t[:, 5 * q:5 * (q + 1), :])
                    nc.sync.dma_start(out=wq_sb, in_=wqT_t)
                    nc.sync.dma_start(out=mask_sb, in_=mask[:])
                    nc.sync.dma_start(out=ones_sb, in_=ones[:])
                    nc.sync.dma_start(out=cos_sb[:, BLK:S], in_=cosT[:, BLK:S])
                    nc.sync.dma_start(out=sin_sb[:, BLK:S], in_=sinT[:, BLK:S])
                    nc.sync.dma_start(out=wo_sb, in_=woT_t)

                # ---- K/V/Q projections, V groups interleaved between K/Q
                # strips so each pv PSUM copy (pvp bufs=1) hides under the
                # following strip's matmuls ----
                def kstrip(kvh, tp, xb):
                    pk = wkpp.tile([128, BLK], F32, tag="ps", name="pk")
                    for a in range(HT):
                        nc.tensor.matmul(pk[:], wk_sb[:, a, kvh * HD:(kvh + 1) * HD],
                                         xb(a), start=(a == 0), stop=(a == HT - 1))
                    rope(kT_sb[:, kvh, tp:tp + BLK], pk[:], tp, BLK)

                def vgroup(tt, bp, xb):
                    pv = pvp.tile([128, KV * HD], F32, tag="pv", name="pv")
                    for a in range(HT):
                        nc.tensor.matmul(pv[:], xb(a, slice(tt * 128, (tt + 1) * 128)),
                                         wv_sb[:, a, :], start=(a == 0), stop=(a == HT - 1))
                    nc.scalar.copy(v_sb[:, bp * (BLK // 128) + tt, :], pv[:])

                def qstrip(h, tp, xb):
                    pq = wkpp.tile([128, BLK], F32, tag="ps", name="pq")
                    for a in range(HT):
                        nc.tensor.matmul(pq[:], wq_sb[:, a, h * HD:(h + 1) * HD],
                                         xb(a), start=(a == 0), stop=(a == HT - 1))
                    rope(qT_sb[:, h, tp:tp + BLK], pq[:], tp, BLK)

                def do_proj(first, bp, xb):
                    tp = bp * BLK
                    if first:
                        # wv/wq are still in flight behind the wk/xb stream:
                        # front-load both K strips (wk+xb only) to keep the PE
                        # busy until wv lands, then V groups until wq lands
                        kstrip(0, tp, xb); kstrip(1, tp, xb); vgroup(0, bp, xb)
                        qstrip(0, tp, xb); vgroup(1, bp, xb); qstrip(1, tp, xb)
                        vgroup(2, bp, xb); qstrip(2, tp, xb); vgroup(3, bp, xb)
                        qstrip(3, tp, xb); qstrip(4, tp, xb)
                    else:
                        kstrip(0, tp, xb); vgroup(0, bp, xb); kstrip(1, tp, xb)
                        vgroup(1, bp, xb); qstrip(0, tp, xb); vgroup(2, bp, xb)
                        qstrip(1, tp, xb); vgroup(3, bp, xb); qstrip(2, tp, xb)
                        qstrip(3, tp, xb); qstrip(4, tp, xb)

                if blk == 0:
                    do_proj(True, 0, xblk)

                # prefetch the next block's x ahead of this chunk's y DMAs
                if blk + 1 < NBLK:
                    xq_next = alloc_xq()
                    load_xq(xq_next, blk + 1)

                # ---- attention chunk for these 512 queries ----
                c = blk
                q0 = c * CH
                ki_max = 4 * c + 3
                at_ch = atp.tile([128, HEADS, CH], BF16, tag="atc")  # 5KB/part
                for h in range(HEADS):
                    kvh = KVIDX[h]
                    pat = patp.tile([128, CH], F32, tag="pat")
                    # softmax row-sum partials accumulate on the DVE (fp32),
                    # final partial written bf16 for a cheap ones-matmul
                    racc = raccp.tile([128, CH], F32, tag="racc")
                    rb = raccp.tile([128, CH], BF16, tag="rb")
                    for ki in range(ki_max + 1):
                        off = max(0, ki * 128 - q0)
                        ps = wkpp.tile([128, CH], F32, tag="ps")
                        nc.tensor.matmul(ps[:, off:CH],
                                         kT_sb[:, kvh, ki * 128:(ki + 1) * 128],
                                         qT_sb[:, h, q0 + off:q0 + CH],
                                         start=True, stop=True)
                        et = etp.tile([128, CH], BF16, tag="et")
                        nc.scalar.activation(out=et[:, off:CH], in_=ps[:, off:CH],
                                             func=mybir.ActivationFunctionType.Exp,
                                             scale=SCALE)
                        if ki >= 4 * c:  # diagonal tile: causal mask
                            nc.vector.tensor_mul(et[:, off:off + 128],
                                                 et[:, off:off + 128], mask_sb[:])
                        if ki == 0:
                            nc.vector.tensor_copy(racc[:], et[:])
                        elif ki == ki_max:
                            # last ki is a diagonal tile (off=384): write the
                            # bf16 partial; cols below off pass through
                            nc.vector.tensor_add(rb[:, off:CH], racc[:, off:CH],
                                                 et[:, off:CH])
                            nc.vector.tensor_copy(rb[:, 0:off], racc[:, 0:off])
                        else:
                            nc.vector.tensor_add(racc[:, off:CH], racc[:, off:CH],
                                                 et[:, off:CH])
                        nc.tensor.matmul(pat[:, off:CH],
                                         v_sb[:, ki, kvh * HD:(kvh + 1) * HD],
                                         et[:, off:CH], start=(ki == 0),
                                         stop=(ki == ki_max))
                    # one partition-reduce matmul per (head, chunk); every
                    # output row holds R so the DVE normalizer mul is
                    # element-wise. 1/R = exp(-ln R) on the ACT engine.
                    pR = prp.tile([128, CH], F32, tag="pR")
                    nc.tensor.matmul(pR[:], ones_sb[:], rb[:],
                                     start=True, stop=True)
                    lr = recp.tile([128, CH], F32, tag="lr")
                    nc.scalar.activation(out=lr[:], in_=pR[:],
                                         func=mybir.ActivationFunctionType.Ln)
                    rec = recp.tile([128, CH], F32, tag="rec")
                    nc.scalar.activation(out=rec[:], in_=lr[:],
                                         func=mybir.ActivationFunctionType.Exp,
                                         scale=-1.0)
                    nc.vector.tensor_mul(at_ch[:, h, :], pat[:], rec[:])

                # next block's projections BEFORE this chunk's o_proj: the PE
                # chews proj strips while the ACT drains this chunk's exps
                if blk + 1 < NBLK:
                    xq = xq_next

                    def xblk_n(a, sl=slice(None), xq_=xq_next):
                        return xq_[a // 5][:, a % 5, sl]

                    do_proj(False, blk + 1, xblk_n)

                # ---- o_proj for this chunk (bf16 out, DVE copies) ----
                for tt in range(CH // 128):
                    for n0 in range(0, NO, 2):
                        nn_ = min(2, NO - n0)
                        yst = cst.tile([128, 2 * CH], BF16, tag="yst")
                        for j in range(nn_):
                            n = n0 + j
                            py = wkpp.tile([128, CH], F32, tag="ps")
                            for h in range(HEADS):
                                nc.tensor.matmul(py[:], at_ch[:, h, tt * 128:(tt + 1) * 128],
                                                 wo_sb[:, h, n * CH:(n + 1) * CH],
                                                 start=(h == 0), stop=(h == HEADS - 1))
                            nc.vector.tensor_copy(yst[:, j * CH:(j + 1) * CH], py[:])
                        nc.sync.dma_start(
                            out=y[q0 + tt * 128:q0 + (tt + 1) * 128,
                                  n0 * CH:(n0 + nn_) * CH],
                            in_=yst[:, 0:nn_ * CH])

    _split_waits(nc)
    nc.finalize()
    return nc


def core_heads(g):
    """Query-head and kv-head global indices for core group g (= core % 4)."""
    qh = [4 * g, 4 * g + 1, 4 * g + 2, 4 * g + 3, 16 + g]
    kvh = [g, 4]
    return qh, kvh


def make_in_maps(hidden_states, position_ids, wq, wk, wv, wo, sub_w):
    hidden_states = np.asarray(hidden_states, dtype=np.float32)
    position_ids = np.asarray(position_ids)
    wq = np.asarray(wq, dtype=np.float32)
    wk = np.asarray(wk, dtype=np.float32)
    wv = np.asarray(wv, dtype=np.float32)
    wo = np.asarray(wo, dtype=np.float32)
    sub_w = np.asarray(sub_w, dtype=np.float32)

    wo_s = wo * sub_w[None, :]          # fold BitNetSubNorm gain into o_proj
    inv_freq = (1.0 / (THETA ** (np.arange(0, HD, 2, dtype=np.float32) / HD)))  # [64]
    mask01 = np.triu(np.ones((128, 128))).astype(ml_dtypes.bfloat16)
    bf = ml_dtypes.bfloat16
    ones128 = np.ones((128, 128), dtype=bf)

    # per-batch tensors (shared by the 4 cores of a batch): compute once
    xT_b, cosT_b, sinT_b = [], [], []
    for b in range(B):
        xT_b.append(np.ascontiguousarray(hidden_states[b].T).astype(bf))  # [HID, S]
        pos = position_ids[b].astype(np.float32)                      # [S]
        ang = inv_freq[:, None] * pos[None, :]                        # [64, S]
        cosT = np.concatenate([np.cos(ang), np.cos(ang)], axis=0)     # [128, S]
        sinT = np.concatenate([-np.sin(ang), np.sin(ang)], axis=0)    # sign-folded
        cosT_b.append(np.ascontiguousarray(cosT).astype(bf))
        sinT_b.append(np.ascontiguousarray(sinT).astype(bf))
    # per-group weight shards (shared by the 2 cores of a group): once
    wshard = []
    for g in range(4):
        qh, kvh = core_heads(g)
        qrows = np.concatenate([np.arange(h * HD, (h + 1) * HD) for h in qh])
        krows = np.concatenate([np.arange(k * HD, (k + 1) * HD) for k in kvh])
        wshard.append({
            "wqT": np.ascontiguousarray(wq[qrows].T).astype(bf),      # [HID, 640]
            "wkT": np.ascontiguousarray(wk[krows].T).astype(bf),      # [HID, 256]
            "wvT": np.ascontiguousarray(wv[krows].T).astype(bf),      # [HID, 256]
            "woT": np.ascontiguousarray(wo_s[:, qrows].T).astype(bf),
        })

    in_maps = []
    for c in range(NCORES):
        b, g = c // 4, c % 4
        in_maps.append({
            "xT": xT_b[b],
            "cosT": cosT_b[b],
            "sinT": sinT_b[b],
            "mask": mask01,
            "ones": ones128,
            **wshard[g],
        })
    return in_maps


def kernel(hidden_states, position_ids, wq, wk, wv, wo, sub_w, _trace=False):
    if "nc" not in _CACHE:
        _CACHE["nc"] = build_nc()
    nc = _CACHE["nc"]
    in_maps = make_in_maps(hidden_states, position_ids, wq, wk, wv, wo, sub_w)
    res = run_bass_kernel_spmd(nc, in_maps, core_ids=list(range(NCORES)), trace=_trace)
    _CACHE["last_results"] = res
    out = np.zeros((B, S, HID), dtype=np.float32)
    for c in range(NCORES):
        out[c // 4] += res.results[c]["y"].astype(np.float32)
    return out



# revision 10
# speedup vs baseline: 1.5115x; 1.1322x over previous
"""BitNetAttention Trainium2 kernel (nn_BitNetAttention, B=2 S=2048 HID=2560).

Reference: q/k/v projections (x @ W^T), RoPE (rotate-half, theta=5e5), causal
GQA attention (20 q heads, 5 kv heads, head_dim 128), BitNetSubNorm per-channel
gain, o_proj.

Sharding across 8 NeuronCores: core c handles batch c//4 and 5 query heads:
with g = c%4, q heads [4g..4g+3, 16+g], kv heads [g, 4]. This grouping makes
the local head->kv map the constant [0,0,0,0,1] so one SPMD program serves all
cores. Each core computes its batch's partial o_proj output (sum over its 5
heads); the host sums 4 partials per batch. sub_w is folded into wo on host.

Precision: everything bf16 with fp32 PSUM accumulation; RoPE in fp32 from
bf16 cos/sin tables; softmax normalizer 1/R computed as exp(-ln R) on the
ACT engine (keeps the slow DVE reciprocal off the per-head critical path;
bass blocks the ACT Reciprocal function and custom-DVE approx ops fail
walrus codegen here).

Structure: one pass over 512-wide x blocks; for each block compute K strips
(RoPE'd, [d,t] layout), V tiles, Q strips (V PSUM groups interleaved between
K/Q strips so their single-buffer copy hides under strip matmuls), then
immediately the attention chunk for the same 512 queries (QK^T in [k,q]
layout -> exp on ACT -> causal mask mul on the diagonal tile -> PV
accumulating in PSUM). Softmax row sums accumulate on the DVE (fp32 SBUF
tile += exp tile per ki, final partial written bf16) with a single
ones-matmul per (head, chunk) reducing over partitions -- this keeps the
per-ki row-sum matmuls off the PE (~36us/core saved vs accumulating R in
PSUM). The NEXT block's projections are issued before this chunk's o_proj
so the PE has proj work while the ACT drains the chunk's exp tail; o_proj
(PSUM shared with QK scores via the 4-deep work pool) stages to bf16 via
DVE and DMAs out 1024 wide. PE ~90% busy. Walrus's 1-sync-wait ISA limit is
handled by a post-pass moving surplus waits onto EventSemaphore
instructions.
"""

import numpy as np
import ml_dtypes
from contextlib import ExitStack

import concourse.bass as bass
import concourse.mybir as mybir
import concourse.tile as tile
from concourse.bass_utils import run_bass_kernel_spmd

F32 = mybir.dt.float32
BF16 = mybir.dt.bfloat16

B, S, HID = 2, 2048, 2560
NH, NKV, HD = 20, 5, 128
G = NH // NKV
THETA = 500000.0
NCORES = 8
HEADS = 5          # query heads per core
KV = 2             # kv heads per core
KVIDX = [0, 0, 0, 0, 1]   # local head -> local kv head
HT = HID // 128    # 20 hidden k-tiles
BLK = 512          # x block width (t) = attention q-chunk width
NBLK = S // BLK    # 4
CH = 512
KT = S // 128      # 16 k-tiles
NO = HID // CH     # 5 o_proj hid chunks
SCALE = HD ** -0.5

_CACHE = {}


def _split_waits(nc):
    """Walrus ISA structs carry a single sync-wait slot. Move surplus waits
    onto EventSemaphore sequencer instructions inserted just before (same
    engine; engines are in-order so hoisting waits earlier is safe)."""
    import concourse.mybir as mb
    n_ev = 0
    for f in nc.m.functions:
        for bb in f.blocks:
            out = []
            changed = False
            for inst in bb.instructions:
                si = getattr(inst, "sync_info", None)
                if (type(inst).__name__ != "InstEventSemaphore" and si is not None
                        and len(si.on_wait) > 1):
                    waits = list(si.on_wait)
                    for w in waits[:-1]:
                        ev = mb.InstEventSemaphore(name=f"I-evw-{n_ev}", ins=[], outs=[])
                        n_ev += 1
                        ev.engine = inst.engine
                        ev.sync_info = mb.SyncInfo(on_wait=[w], on_update=[])
                        nc.register_instruction(ev)
                        out.append(ev)
                    inst.sync_info = mb.SyncInfo(on_wait=waits[-1:],
                                                 on_update=list(si.on_update))
                    changed = True
                out.append(inst)
            if changed:
                bb.instructions = out
    return n_ev


def build_nc(reps=1):
    nc = bass.Bass()
    xT = nc.declare_dram_parameter("xT", [HID, S], BF16, isOutput=False)
    wqT = nc.declare_dram_parameter("wqT", [HID, HEADS * HD], BF16, isOutput=False)
    wkT = nc.declare_dram_parameter("wkT", [HID, KV * HD], BF16, isOutput=False)
    wvT = nc.declare_dram_parameter("wvT", [HID, KV * HD], BF16, isOutput=False)
    woT = nc.declare_dram_parameter("woT", [HEADS * HD, HID], BF16, isOutput=False)
    cosT = nc.declare_dram_parameter("cosT", [HD, S], BF16, isOutput=False)
    sinT = nc.declare_dram_parameter("sinT", [HD, S], BF16, isOutput=False)  # sign-folded
    mask = nc.declare_dram_parameter("mask", [128, 128], BF16, isOutput=False)  # triu 0/1
    ones = nc.declare_dram_parameter("ones", [128, 128], BF16, isOutput=False)
    y = nc.declare_dram_parameter("y", [S, HID], BF16, isOutput=True)

    xT_t = xT.rearrange("(a p) t -> p a t", p=128)      # [128, 20, 2048]
    wqT_t = wqT.rearrange("(a p) d -> p a d", p=128)    # [128, 20, 640]
    wkT_t = wkT.rearrange("(a p) d -> p a d", p=128)
    wvT_t = wvT.rearrange("(a p) d -> p a d", p=128)
    woT_t = woT.rearrange("(h p) n -> p h n", p=128)    # [128, 5, 2560]

    with tile.TileContext(nc) as tc:
      for rep in range(reps):
        with ExitStack() as ctx:
            # ---------- persistent tiles ----------
            per = ctx.enter_context(tc.tile_pool(name=f"persist{rep}", bufs=1))
            kT_sb = per.tile([128, KV, S], BF16)        # 8KB/part
            v_sb = per.tile([128, KT, KV * HD], BF16)   # 8KB/part [t%128, ktile, kv*128+d]
            cos_sb = per.tile([HD, S], BF16)            # 4KB
            sin_sb = per.tile([HD, S], BF16)            # 4KB
            mask_sb = per.tile([128, 128], BF16)
            ones_sb = per.tile([128, 128], BF16)
            qT_sb = per.tile([128, HEADS, S], BF16)     # 20KB/part

            wp = ctx.enter_context(tc.tile_pool(name=f"w{rep}", bufs=1))
            wk_sb = wp.tile([128, HT, KV * HD], BF16)   # 10KB/part
            wv_sb = wp.tile([128, HT, KV * HD], BF16)   # 10KB/part
            wq_sb = wp.tile([128, HT, HEADS * HD], BF16)  # 25.6KB/part
            wo_sb = wp.tile([128, HEADS, HID], BF16)    # 25.6KB/part

            xp = ctx.enter_context(tc.tile_pool(name=f"x{rep}", bufs=2))
            ropep = ctx.enter_context(tc.tile_pool(name=f"rope{rep}", bufs=1))
            # PSUM pools (8 banks total): work pool (proj strips + QK
            # scores + o_proj py) 4 banks, pv 1, pat 2, pR 1
            wkpp = ctx.enter_context(tc.tile_pool(name=f"pw{rep}", bufs=4, space="PSUM"))
            pvp = ctx.enter_context(tc.tile_pool(name=f"pv{rep}", bufs=1, space="PSUM"))
            patp = ctx.enter_context(tc.tile_pool(name=f"pat{rep}", bufs=2, space="PSUM"))
            prp = ctx.enter_context(tc.tile_pool(name=f"pr{rep}", bufs=1, space="PSUM"))
            etp = ctx.enter_context(tc.tile_pool(name=f"et{rep}", bufs=6))
            recp = ctx.enter_context(tc.tile_pool(name=f"rec{rep}", bufs=2))
            raccp = ctx.enter_context(tc.tile_pool(name=f"racc{rep}", bufs=2))
            atp = ctx.enter_context(tc.tile_pool(name=f"at{rep}", bufs=2))
            cst = ctx.enter_context(tc.tile_pool(name=f"yst{rep}", bufs=3))

            def rope(dst, psrc, t0, w):
                t1 = ropep.tile([128, BLK], F32, tag="t1")
                t2 = ropep.tile([128, BLK], F32, tag="t2")
                nc.vector.tensor_mul(t1[:, 0:w], psrc, cos_sb[:, t0:t0 + w])
                nc.vector.tensor_mul(t2[0:64, 0:w], psrc[64:128, :], sin_sb[0:64, t0:t0 + w])
                nc.vector.tensor_mul(t2[64:128, 0:w], psrc[0:64, :], sin_sb[64:128, t0:t0 + w])
                nc.vector.tensor_add(dst, t1[:, 0:w], t2[:, 0:w])

            # xblk as 4 independent tiles so the first K matmuls only wait
            # on the first 5 a-tiles, not the whole block. blk0's tiles load
            # inline (interleaved with wk); later blocks are prefetched at
            # the end of the PREVIOUS block's projections, ahead of that
            # block's y-output DMAs on the Sync queue.
            def alloc_xq():
                return [xp.tile([128, 5, BLK], BF16, tag=f"xb{q}", name=f"xb{q}")
                        for q in range(4)]

            def load_xq(tiles, blk_):
                tb = blk_ * BLK
                for q in range(4):
                    nc.sync.dma_start(out=tiles[q],
                                      in_=xT_t[:, 5 * q:5 * (q + 1), tb:tb + BLK])

            xq = None
            for blk in range(NBLK):
                t0 = blk * BLK
                if blk == 0:
                    xq = alloc_xq()
                    for q in range(4):
                        if q == 0:
                            # finest-grain first tiles so the first K matmul
                            # can start as early as possible
                            nc.sync.dma_start(out=wk_sb[:, 0:1, :], in_=wkT_t[:, 0:1, :])
                            nc.sync.dma_start(out=xq[0][:, 0:1, :],
                                              in_=xT_t[:, 0:1, t0:t0 + BLK])
                            nc.sync.dma_start(out=wk_sb[:, 1:5, :], in_=wkT_t[:, 1:5, :])
                            nc.sync.dma_start(out=xq[0][:, 1:5, :],
                                              in_=xT_t[:, 1:5, t0:t0 + BLK])
                            continue
                        nc.sync.dma_start(out=wk_sb[:, 5 * q:5 * (q + 1), :],
                                          in_=wkT_t[:, 5 * q:5 * (q + 1), :])
                        nc.sync.dma_start(out=xq[q],
                                          in_=xT_t[:, 5 * q:5 * (q + 1), t0:t0 + BLK])
                        if q == 1:
                            # in time for the first K strip's RoPE, but out of
                            # the way of the a<=9 portion of the chain
                            nc.sync.dma_start(out=cos_sb[:, 0:BLK], in_=cosT[:, 0:BLK])
                            nc.sync.dma_start(out=sin_sb[:, 0:BLK], in_=sinT[:, 0:BLK])

                def xblk(a, sl=slice(None), xq_=xq):
                    return xq_[a // 5][:, a % 5, sl]

                if blk == 0:
                    for q in range(4):
                        nc.sync.dma_start(out=wv_sb[:, 5 * q:5 * (q + 1), :],
                                          in_=wvT_t[:, 5 * q:5 * (q + 1), :])
                    nc.sync.dma_start(out=wq_sb, in_=wqT_t)
                    nc.sync.dma_start(out=mask_sb, in_=mask[:])
                    nc.sync.dma_start(out=ones_sb, in_=ones[:])
                    nc.sync.dma_start(out=cos_sb[:, BLK:S], in_=cosT[:, BLK:S])
                    nc.sync.dma_start(out=sin_sb[:, BLK:S], in_=sinT[:, BLK:S])
                    nc.sync.dma_start(out=wo_sb, in_=woT_t)

                # ---- K/V/Q projections, V groups interleaved between K/Q
                # strips so each pv PSUM copy (pvp bufs=1) hides under the
                # following strip's matmuls ----
                def kstrip(kvh, tp, xb):
                    pk = wkpp.tile([128, BLK], F32, tag="ps", name="pk")
                    for a in range(HT):
                        nc.tensor.matmul(pk[:], wk_sb[:, a, kvh * HD:(kvh + 1) * HD],
                                         xb(a), start=(a == 0), stop=(a == HT - 1))
                    rope(kT_sb[:, kvh, tp:tp + BLK], pk[:], tp, BLK)

                def vgroup(tt, bp, xb):
                    pv = pvp.tile([128, KV * HD], F32, tag="pv", name="pv")
                    for a in range(HT):
                        nc.tensor.matmul(pv[:], xb(a, slice(tt * 128, (tt + 1) * 128)),
                                         wv_sb[:, a, :], start=(a == 0), stop=(a == HT - 1))
                    nc.scalar.copy(v_sb[:, bp * (BLK // 128) + tt, :], pv[:])

                def qstrip(h, tp, xb):
                    pq = wkpp.tile([128, BLK], F32, tag="ps", name="pq")
                    for a in range(HT):
                        nc.tensor.matmul(pq[:], wq_sb[:, a, h * HD:(h + 1) * HD],
                                         xb(a), start=(a == 0), stop=(a == HT - 1))
                    rope(qT_sb[:, h, tp:tp + BLK], pq[:], tp, BLK)

                def do_proj(first, bp, xb):
                    tp = bp * BLK
                    if first:
                        # wv/wq are still in flight behind the wk/xb stream:
                        # front-load both K strips (wk+xb only) to keep the PE
                        # busy until wv lands, then V groups until wq lands
                        kstrip(0, tp, xb); kstrip(1, tp, xb); vgroup(0, bp, xb)
                        qstrip(0, tp, xb); vgroup(1, bp, xb); qstrip(1, tp, xb)
                        vgroup(2, bp, xb); qstrip(2, tp, xb); vgroup(3, bp, xb)
                        qstrip(3, tp, xb); qstrip(4, tp, xb)
                    else:
                        kstrip(0, tp, xb); vgroup(0, bp, xb); kstrip(1, tp, xb)
                        vgroup(1, bp, xb); qstrip(0, tp, xb); vgroup(2, bp, xb)
                        qstrip(1, tp, xb); vgroup(3, bp, xb); qstrip(2, tp, xb)
                        qstrip(3, tp, xb); qstrip(4, tp, xb)

                if blk == 0:
                    do_proj(True, 0, xblk)

                # prefetch the next block's x ahead of this chunk's y DMAs
                if blk + 1 < NBLK:
                    xq_next = alloc_xq()
                    load_xq(xq_next, blk + 1)

                # ---- attention chunk for these 512 queries ----
                c = blk
                q0 = c * CH
                ki_max = 4 * c + 3
                at_ch = atp.tile([128, HEADS, CH], BF16, tag="atc")  # 5KB/part
                for h in range(HEADS):
                    kvh = KVIDX[h]
                    pat = patp.tile([128, CH], F32, tag="pat")
                    # softmax row-sum partials accumulate on the DVE in bf16
                    # (2x-1p rate; partition-reduce matmul re-sums in fp32)
                    racc = raccp.tile([128, CH], BF16, tag="racc")
                    for ki in range(ki_max + 1):
                        off = max(0, ki * 128 - q0)
                        ps = wkpp.tile([128, CH], F32, tag="ps")
                        nc.tensor.matmul(ps[:, off:CH],
                                         kT_sb[:, kvh, ki * 128:(ki + 1) * 128],
                                         qT_sb[:, h, q0 + off:q0 + CH],
                                         start=True, stop=True)
                        et = etp.tile([128, CH], BF16, tag="et")
                        if ki >= 4 * c:
                            # diagonal tile: exp the 128-wide diag block first
                            # so its causal mask mul (DVE) overlaps the exp of
                            # the remaining columns (ACT)
                            nc.scalar.activation(out=et[:, off:off + 128],
                                                 in_=ps[:, off:off + 128],
                                                 func=mybir.ActivationFunctionType.Exp,
                                                 scale=SCALE)
                            nc.vector.tensor_mul(et[:, off:off + 128],
                                                 et[:, off:off + 128], mask_sb[:])
                            if CH - off > 128:
                                nc.scalar.activation(
                                    out=et[:, off + 128:CH],
                                    in_=ps[:, off + 128:CH],
                                    func=mybir.ActivationFunctionType.Exp,
                                    scale=SCALE)
                        else:
                            nc.scalar.activation(out=et[:, off:CH], in_=ps[:, off:CH],
                                                 func=mybir.ActivationFunctionType.Exp,
                                                 scale=SCALE)
                        if ki == 0:
                            nc.vector.tensor_copy(racc[:], et[:])
                        else:
                            nc.vector.tensor_add(racc[:, off:CH], racc[:, off:CH],
                                                 et[:, off:CH])
                        nc.tensor.matmul(pat[:, off:CH],
                                         v_sb[:, ki, kvh * HD:(kvh + 1) * HD],
                                         et[:, off:CH], start=(ki == 0),
                                         stop=(ki == ki_max))
                    # one partition-reduce matmul per (head, chunk); every
                    # output row holds R so the DVE normalizer mul is
                    # element-wise. 1/R = exp(-ln R) on the ACT engine.
                    pR = prp.tile([128, CH], F32, tag="pR")
                    nc.tensor.matmul(pR[:], ones_sb[:], racc[:],
                                     start=True, stop=True)
                    lr = recp.tile([128, CH], F32, tag="lr")
                    nc.scalar.activation(out=lr[:], in_=pR[:],
                                         func=mybir.ActivationFunctionType.Ln)
                    rec = recp.tile([128, CH], F32, tag="rec")
                    nc.scalar.activation(out=rec[:], in_=lr[:],
                                         func=mybir.ActivationFunctionType.Exp,
                                         scale=-1.0)
                    nc.vector.tensor_mul(at_ch[:, h, :], pat[:], rec[:])

                # next block's projections BEFORE this chunk's o_proj: the PE
                # chews proj strips while the ACT drains this chunk's exps
                if blk + 1 < NBLK:
                    xq = xq_next

                    def xblk_n(a, sl=slice(None), xq_=xq_next):
                        return xq_[a // 5][:, a % 5, sl]

                    do_proj(False, blk + 1, xblk_n)

                # ---- o_proj for this chunk (bf16 out, DVE copies) ----
                for tt in range(CH // 128):
                    for n0 in range(0, NO, 2):
                        nn_ = min(2, NO - n0)
                        yst = cst.tile([128, 2 * CH], BF16, tag="yst")
                        for j in range(nn_):
                            n = n0 + j
                            py = wkpp.tile([128, CH], F32, tag="ps")
                            for h in range(HEADS):
                                nc.tensor.matmul(py[:], at_ch[:, h, tt * 128:(tt + 1) * 128],
                                                 wo_sb[:, h, n * CH:(n + 1) * CH],
                                                 start=(h == 0), stop=(h == HEADS - 1))
                            # ACT stages y (PSUM->SBUF bf16): keeps the DVE
                            # free for RoPE + row-sum adds
                            nc.scalar.copy(yst[:, j * CH:(j + 1) * CH], py[:])
                        nc.sync.dma_start(
                            out=y[q0 + tt * 128:q0 + (tt + 1) * 128,
                                  n0 * CH:(n0 + nn_) * CH],
                            in_=yst[:, 0:nn_ * CH])

    _split_waits(nc)
    nc.finalize()
    return nc


def core_heads(g):
    """Query-head and kv-head global indices for core group g (= core % 4)."""
    qh = [4 * g, 4 * g + 1, 4 * g + 2, 4 * g + 3, 16 + g]
    kvh = [g, 4]
    return qh, kvh


def make_in_maps(hidden_states, position_ids, wq, wk, wv, wo, sub_w):
    hidden_states = np.asarray(hidden_states, dtype=np.float32)
    position_ids = np.asarray(position_ids)
    wq = np.asarray(wq, dtype=np.float32)
    wk = np.asarray(wk, dtype=np.float32)
    wv = np.asarray(wv, dtype=np.float32)
    wo = np.asarray(wo, dtype=np.float32)
    sub_w = np.asarray(sub_w, dtype=np.float32)

    wo_s = wo * sub_w[None, :]          # fold BitNetSubNorm gain into o_proj
    inv_freq = (1.0 / (THETA ** (np.arange(0, HD, 2, dtype=np.float32) / HD)))  # [64]
    mask01 = np.triu(np.ones((128, 128))).astype(ml_dtypes.bfloat16)
    bf = ml_dtypes.bfloat16
    ones128 = np.ones((128, 128), dtype=bf)

    # per-batch tensors (shared by the 4 cores of a batch): compute once
    xT_b, cosT_b, sinT_b = [], [], []
    for b in range(B):
        xT_b.append(np.ascontiguousarray(hidden_states[b].T).astype(bf))  # [HID, S]
        pos = position_ids[b].astype(np.float32)                      # [S]
        ang = inv_freq[:, None] * pos[None, :]                        # [64, S]
        cosT = np.concatenate([np.cos(ang), np.cos(ang)], axis=0)     # [128, S]
        sinT = np.concatenate([-np.sin(ang), np.sin(ang)], axis=0)    # sign-folded
        cosT_b.append(np.ascontiguousarray(cosT).astype(bf))
        sinT_b.append(np.ascontiguousarray(sinT).astype(bf))
    # per-group weight shards (shared by the 2 cores of a group): once
    wshard = []
    for g in range(4):
        qh, kvh = core_heads(g)
        qrows = np.concatenate([np.arange(h * HD, (h + 1) * HD) for h in qh])
        krows = np.concatenate([np.arange(k * HD, (k + 1) * HD) for k in kvh])
        wshard.append({
            "wqT": np.ascontiguousarray(wq[qrows].T).astype(bf),      # [HID, 640]
            "wkT": np.ascontiguousarray(wk[krows].T).astype(bf),      # [HID, 256]
            "wvT": np.ascontiguousarray(wv[krows].T).astype(bf),      # [HID, 256]
            "woT": np.ascontiguousarray(wo_s[:, qrows].T).astype(bf),
        })

    in_maps = []
    for c in range(NCORES):
        b, g = c // 4, c % 4
        in_maps.append({
            "xT": xT_b[b],
            "cosT": cosT_b[b],
            "sinT": sinT_b[b],
            "mask": mask01,
            "ones": ones128,
            **wshard[g],
        })
    return in_maps


def kernel(hidden_states, position_ids, wq, wk, wv, wo, sub_w, _trace=False):
    if "nc" not in _CACHE:
        _CACHE["nc"] = build_nc()
    nc = _CACHE["nc"]
    in_maps = make_in_maps(hidden_states, position_ids, wq, wk, wv, wo, sub_w)
    res = run_bass_kernel_spmd(nc, in_maps, core_ids=list(range(NCORES)), trace=_trace)
    _CACHE["last_results"] = res
    out = np.zeros((B, S, HID), dtype=np.float32)
    for c in range(NCORES):
        out[c // 4] += res.results[c]["y"].astype(np.float32)
    return out

